# revision 7
# baseline (speedup 1.0000x reference)
"""Trainium2 Bass kernel for nn_Attention_89172110999574.

Strategy (8 NeuronCores, data parallel — 1 batch element per core):
  - Scores computed TRANSPOSED (ST[j,i] = k_j . q_i), operands bf16.
  - attn@V swapped: lhsT = exp-scores slice [128 j, 128 i] (stationary),
    rhs = [v | 1] [128 j, 65] -> out OT[i-slice, dv|den] in PSUM. N=65 per
    matmul instead of 512 -> attnV PE cost drops ~4x; softmax denominator
    rides as output column 64.
  - Relative-position bias is block-Toeplitz. Two application paths balanced
    across engines:
      * type-A heads (NB..7): identity-matmul adds raw bias strips into the
        score PSUM (spare PE capacity).
      * type-B heads (0..NB-1): exp(bias) table multiplied into exp(scores)
        on DVE (exp(a+b) = exp(a)*exp(b)).
    Strip tables (bf16) built by strided DRAM->SBUF DMAs, two heads per DMA,
    ordered so each head's strips land just before that head runs.
  - The whole (head, jt) space runs as one flat software pipeline: attnV
    trails scores/exp by EXP_LAG stages ACROSS head boundaries, so neither
    PE nor ACT stalls at head transitions.
  - Normalization: per-partition reciprocal of OT column 64 (DVE), applied
    by Pool tensor_scalar into og[i, is, h', dv]; per-head gelu on ACT;
    each head-pair block moved to phase-E layout by ONE hardware
    DMA-transpose (out[p,m,l] = in[l, 128m+p]).
  - Output projection + BatchNorm affine fused on DVE.
"""

import os
import sys

import numpy as np

for _p in ("/opt/trn_rl_repo", "/root/.axon_site/_ro/trn_rl_repo"):
    if os.path.isdir(_p) and _p not in sys.path:
        sys.path.insert(0, _p)

import concourse.bass as bass
import concourse.tile as tile
from concourse import mybir
from concourse.bass_utils import run_bass_kernel_spmd
from concourse.masks import make_identity

N = 1024          # tokens per batch (32*32)
D = 256           # model dim
H = 8             # heads
DK = 32           # head dim (qk)
DV = 64           # head dim (v)
DOUT = 256        # output dim
NCORES = 8
FM = 32           # fmap
SCALE = float(DK) ** -0.5          # 1/sqrt(32)
RS32 = float(np.sqrt(32.0))        # bias premultiplier: bias/scale = sqrt(32)*E
BN_C = float(1.0 / np.sqrt(1.0 + 1e-5))
F32 = mybir.dt.float32
F32R = mybir.dt.float32r
BF16 = mybir.dt.bfloat16

# heads 0..NB-1 use the exp(bias)-multiply path (DVE); heads NB..7 use
# PE identity-matmul bias adds. Balances PE vs ACT vs DVE busy time.
NB = 4
NA = H - NB
EXP_LAG = 2  # attnV trails scores by this many pipeline stages


def build_nc():
    nc = bass.Bass("TRN2", target_bir_lowering=False, debug=False)

    # x declared f32r (bit-identical to f32) so transposes run all-f32r
    x = nc.dram_tensor("x", [N, D], F32R, kind="ExternalInput").ap()
    wq = nc.dram_tensor("wq", [D, H * DK], F32, kind="ExternalInput").ap()
    wk = nc.dram_tensor("wk", [D, H * DK], F32, kind="ExternalInput").ap()
    wv = nc.dram_tensor("wv", [D, H * DV], F32, kind="ExternalInput").ap()
    wo = nc.dram_tensor("wo", [H * DV, DOUT], F32, kind="ExternalInput").ap()
    pe = nc.dram_tensor("pe", [N, H], F32, kind="ExternalInput").ap()
    bo = nc.dram_tensor("bo", [DOUT], F32, kind="ExternalInput").ap()
    gam = nc.dram_tensor("gam", [DOUT], F32, kind="ExternalInput").ap()
    bet = nc.dram_tensor("bet", [DOUT], F32, kind="ExternalInput").ap()
    out = nc.dram_tensor("out", [N, DOUT], F32, kind="ExternalOutput").ap()

    # scratch DRAM: per-head |s-31| expansion of pos_emb rows (both bf16)
    # wfa: raw values (type-A heads); wfb: exp(sqrt32*E) (type-B heads)
    wfa = nc.dram_tensor("wfa", [NA, 32, 63], BF16).ap()
    wfb = nc.dram_tensor("wfb", [NB, 32, 63], BF16).ap()

    with tile.TileContext(nc) as tc:
        with (
            tc.tile_pool(name="const", bufs=1) as constp,
            tc.tile_pool(name="big", bufs=1) as bigp,
            tc.tile_pool(name="xin", bufs=2) as xinp,
            tc.tile_pool(name="exps", bufs=4) as expp,
            tc.tile_pool(name="exps2", bufs=3) as exp2p,
            tc.tile_pool(name="small", bufs=1) as smallp,
            tc.tile_pool(name="yout", bufs=3) as youtp,
            tc.tile_pool(name="ps2", bufs=2, space="PSUM") as ps2p,
        ):
            # -------- input DMAs first: x (scalar q) + weights (sync q) ----
            xa = []
            for half in range(2):
                xt_in = xinp.tile([128, 4, D], F32R, tag=f"xa{half}", bufs=1,
                                  name=f"xa{half}")
                eng = nc.scalar if half == 0 else nc.sync
                eng.dma_start(
                    out=xt_in,
                    in_=bass.AP(tensor=x.tensor, offset=half * 512 * D,
                                ap=[[D, 128], [128 * D, 4], [1, D]]))
                xa.append(xt_in)
            wq_sb = constp.tile([128, 2, 256], BF16)
            wk_sb = constp.tile([128, 2, 256], BF16)
            wv_sb = constp.tile([128, 2, 512], BF16)
            wo_sb = constp.tile([128, 4, 256], BF16)
            wstgs = []
            for i, (wsrc, nk, cols) in enumerate(
                    ((wq, 2, 256), (wk, 2, 256), (wv, 2, 512), (wo, 4, 256))):
                wstg = xinp.tile([128, nk, cols], F32, tag=f"wstg{i}", bufs=1,
                                 name=f"wstg{i}")
                nc.sync.dma_start(
                    out=wstg,
                    in_=bass.AP(tensor=wsrc.tensor, offset=0,
                                ap=[[cols, 128], [128 * cols, nk], [1, cols]]))
                wstgs.append(wstg)

            # -------- identity (Pool) ------------------------------------
            ident = constp.tile([128, 128], F32)
            make_identity(nc, ident)
            identr = constp.tile([128, 128], F32R)
            nc.scalar.activation(identr, ident,
                                 mybir.ActivationFunctionType.Copy)

            # -------- pos-emb row tables ---------------------------------
            e_sb = smallp.tile([32, 32, 8], F32, tag="e_sb")
            nc.sync.dma_start(
                out=e_sb,
                in_=bass.AP(tensor=pe.tensor, offset=0,
                            ap=[[32 * H, 32], [H, 32], [1, 32 * H // 32]]),
            )
            # s-flip on DVE: wrow[a, h, s] = E[a, |s-31|, h]
            wrow = smallp.tile([32, 8, 63], F32, tag="wrow")
            nc.vector.tensor_copy(
                wrow[:, :, 0:31],
                bass.AP(tensor=e_sb.tensor, offset=e_sb.offset + 31 * 8,
                        ap=[e_sb.ap[0], [1, 8], [-8, 31]]),
            )
            nc.vector.tensor_copy(
                wrow[:, :, 31:63],
                bass.AP(tensor=e_sb.tensor, offset=e_sb.offset,
                        ap=[e_sb.ap[0], [1, 8], [8, 32]]),
            )
            # type-B rows: exp(sqrt32 * E) bf16; type-A rows: raw bf16
            ewrow = smallp.tile([32, NB, 63], BF16, tag="ewrow")
            nc.scalar.activation(ewrow, wrow[:, 0:NB, :],
                                 mybir.ActivationFunctionType.Exp,
                                 scale=RS32)
            wrowb = smallp.tile([32, NA, 63], BF16, tag="wrowb")
            nc.scalar.activation(wrowb, wrow[:, NB:H, :],
                                 mybir.ActivationFunctionType.Copy)
            nc.scalar.dma_start(out=wfb, in_=ewrow)
            nc.scalar.dma_start(out=wfa, in_=wrowb)

            # strip tables: table[32g+cj, h, u, ci] = row_h[|u-31-g|, |ci-cj|]
            # built 2 heads per DMA (4D in-AP), per (g, asc/desc half),
            # ordered so head pairs land in processing order.
            emB = bigp.tile([128, NB, 66, 32], BF16)
            msA = bigp.tile([128, NA, 66, 32], BF16)

            def fill_strip(dst, src_t, h, engs):
                # ascending halves first: the jt=0 slice only needs u >= 31
                for g in range(4):
                    engs[g % len(engs)].dma_start(
                        out=dst[32 * g:32 * (g + 1), h, 31 + g:63 + g, :],
                        in_=bass.AP(tensor=src_t, offset=2016 * h + 31,
                                    ap=[[-1, 32], [63, 32], [1, 32]]),
                    )
                for g in range(4):
                    engs[(g + 1) % len(engs)].dma_start(
                        out=dst[32 * g:32 * (g + 1), h, g:31 + g, :],
                        in_=bass.AP(tensor=src_t,
                                    offset=2016 * h + 31 * 63 + 31,
                                    ap=[[-1, 32], [-63, 31], [1, 32]]),
                    )

            for hb in range(NB):
                fill_strip(emB, wfb.tensor, hb,
                           [nc.scalar, nc.sync] if hb == 0 else [nc.sync])
            for ha in range(NA):
                fill_strip(msA, wfa.tensor, ha,
                           [nc.sync] if ha < 2 else [nc.gpsimd])

            # -------- weight casts + BN rows (Pool / sync) ---------------
            for dst_sb, wstg in zip((wq_sb, wk_sb, wv_sb, wo_sb), wstgs):
                nc.gpsimd.tensor_copy(dst_sb, wstg)
            g2b = constp.tile([128, DOUT], F32)
            b2b = constp.tile([128, DOUT], F32)
            tmpb = constp.tile([128, DOUT], F32)
            nc.sync.dma_start(
                out=g2b, in_=bass.AP(tensor=gam.tensor, offset=0,
                                     ap=[[0, 128], [1, DOUT]]))
            nc.sync.dma_start(
                out=b2b, in_=bass.AP(tensor=bet.tensor, offset=0,
                                     ap=[[0, 128], [1, DOUT]]))
            nc.sync.dma_start(
                out=tmpb, in_=bass.AP(tensor=bo.tensor, offset=0,
                                      ap=[[0, 128], [1, DOUT]]))
            nc.scalar.mul(g2b, g2b, BN_C)
            nc.vector.tensor_mul(tmpb, tmpb, g2b)
            nc.vector.tensor_add(b2b, b2b, tmpb)

            # v augmented with a ones column per head for the softmax
            # denominator (becomes OT column 64)
            va = bigp.tile([128, 8, 8, 65], BF16)
            nc.scalar.activation(va[:, :, :, 64:65],
                                 ident[:, 0:64],
                                 mybir.ActivationFunctionType.Copy,
                                 bias=1.0, scale=0.0)

            xT = bigp.tile([128, 2, N], BF16)
            qT = bigp.tile([128, 2, N], BF16)
            kT = bigp.tile([128, 2, N], BF16)

            with tc.tile_pool(name="psAB", bufs=1, space="PSUM") as psABp:
                # ------------- phase A: x -> xT -------------------------
                for half in range(2):
                    for sub in range(4):
                        nt = 4 * half + sub
                        for dt_ in range(2):
                            pst = psABp.tile([128, 128], F32R, tag="psab",
                                             bufs=2)
                            nc.tensor.transpose(
                                pst,
                                xa[half][:, sub, 128 * dt_:128 * (dt_ + 1)],
                                identr)
                            nc.vector.tensor_copy(
                                xT[:, dt_, 128 * nt:128 * (nt + 1)], pst)

                # ------------- phase B: QKV proj ------------------------
                for dst_sb, w_sb in ((qT, wq_sb), (kT, wk_sb)):
                    for mt in range(2):
                        for ic in range(2):
                            ps = psABp.tile([128, 512], F32, tag="psb",
                                            bufs=2)
                            for kt in range(2):
                                nc.tensor.matmul(
                                    ps,
                                    w_sb[:, kt, 128 * mt:128 * (mt + 1)],
                                    xT[:, kt, 512 * ic:512 * (ic + 1)],
                                    start=(kt == 0), stop=(kt == 1),
                                )
                            nc.vector.tensor_copy(
                                dst_sb[:, mt, 512 * ic:512 * (ic + 1)], ps)
                for jt in range(8):
                    ps = psABp.tile([128, 512], F32, tag="psb", bufs=2)
                    for kt in range(2):
                        nc.tensor.matmul(
                            ps,
                            xT[:, kt, 128 * jt:128 * (jt + 1)],
                            wv_sb[:, kt, :],
                            start=(kt == 0), stop=(kt == 1),
                        )
                    psr = ps.rearrange("p (h v) -> p h v", v=64)
                    nc.gpsimd.tensor_copy(va[:, jt, :, 0:64], psr)

            # identity scaled by 32 = 1/scale^2: folds bias/scale into the
            # PSUM bias add (bf16; only type-A heads use it)
            isc = constp.tile([128, 128], BF16)
            nc.scalar.mul(isc, ident, float(DK))

            # ---------------- phase C: flat (head, jt) pipeline ----------
            # og per head-pair: [128 i-low, 8 is, 2 h', 64 dv] bf16; after
            # gelu, ONE dma transpose -> gT band [128 hv, 8 is, 128 i-low]
            ogt = []
            gtb = []
            for p in range(4):
                og_p = bigp.tile([128, 8, 2, 64], BF16, tag=f"og{p}",
                                 name=f"og{p}")
                gt_p = bigp.tile([128, 8, 128], BF16, tag=f"gt{p}",
                                 name=f"gt{p}")
                ogt.append(og_p)
                gtb.append(gt_p)
            rd = constp.tile([128, 8, 8], F32)  # reciprocal denominators

            NSTAGE = H * 8
            es_q = [None] * NSTAGE   # attnV lhsT operand per stage
            ot_tiles = {}            # head -> (ota, otb)

            with tc.tile_pool(name="otp", bufs=4, space="PSUM") as otp:
                for s in range(NSTAGE + EXP_LAG):
                    if s < NSTAGE:
                        h, jt = divmod(s, 8)
                        typeB = h < NB
                        mtk = h // 4
                        pb = 32 * (h % 4)
                        ps = ps2p.tile([128, 1024], F32, tag="st")
                        for ic in range(2):
                            nc.tensor.matmul(
                                ps[:, 512 * ic:512 * (ic + 1)],
                                kT[pb:pb + 32, mtk, 128 * jt:128 * (jt + 1)],
                                qT[pb:pb + 32, mtk, 512 * ic:512 * (ic + 1)],
                                start=True, stop=typeB,
                                tile_position=(pb, 0),
                            )
                            if not typeB:
                                u0 = 16 * ic + 31 - 4 * jt
                                nc.tensor.matmul(
                                    ps[:, 512 * ic:512 * (ic + 1)],
                                    isc,
                                    msA[:, h - NB, u0:u0 + 16, :],
                                    start=False, stop=True,
                                )
                        es = expp.tile([128, 1024], BF16, tag="es")
                        nc.scalar.activation(
                            es, ps, mybir.ActivationFunctionType.Exp,
                            scale=SCALE)
                        if typeB:
                            es2 = exp2p.tile([128, 32, 32], BF16, tag="es2")
                            nc.vector.tensor_mul(
                                es2,
                                es.rearrange("p (a b) -> p a b", b=32),
                                emB[:, h, 31 - 4 * jt:63 - 4 * jt, :],
                            )
                            es_q[s] = es2.rearrange("p a b -> p (a b)")
                        else:
                            es_q[s] = es

                    # attnV trails by EXP_LAG stages (across head boundaries)
                    sv = s - EXP_LAG
                    if sv >= 0:
                        hv, jv = divmod(sv, 8)
                        if jv == 0:
                            ota = otp.tile([128, 4, 65], F32, tag="ot",
                                           name=f"ota{hv}")
                            otb = otp.tile([128, 4, 65], F32, tag="ot",
                                           name=f"otb{hv}")
                            ot_tiles[hv] = (ota, otb)
                        ota, otb = ot_tiles[hv]
                        esv = es_q[sv]
                        for isl in range(8):
                            ot = ota if isl < 4 else otb
                            nc.tensor.matmul(
                                ot[:, isl % 4, :],
                                esv[:, 128 * isl:128 * (isl + 1)],
                                va[:, jv, hv, :],
                                start=(jv == 0), stop=(jv == 7),
                            )
                        if jv == 7:
                            # drain head hv: reciprocal + Pool normalize +
                            # per-head gelu; dma-transpose per pair
                            pr = hv // 2
                            hq = hv % 2
                            nc.vector.reciprocal(
                                rd[:, hv, 0:4],
                                ota[:, :, 64:65].rearrange("p a b -> p (a b)"))
                            nc.vector.reciprocal(
                                rd[:, hv, 4:8],
                                otb[:, :, 64:65].rearrange("p a b -> p (a b)"))
                            for isl in range(8):
                                ot = ota if isl < 4 else otb
                                nc.gpsimd.tensor_scalar_mul(
                                    ogt[pr][:, isl, hq, :],
                                    ot[:, isl % 4, 0:64],
                                    rd[:, hv, isl:isl + 1],
                                )
                            ogh = ogt[pr][:, :, hq, :]
                            nc.scalar.activation(
                                ogh, ogh, mybir.ActivationFunctionType.Gelu)
                            if hq == 1:
                                og2 = ogt[pr].rearrange(
                                    "p a b c -> p (a b c)")
                                nc.sync.dma_start_transpose(gtb[pr], og2)

                # ------------- phase E: out proj + BN -------------------
                for it in range(8):
                    ps = ps2p.tile([128, 1024], F32, tag="st")
                    for kt in range(4):
                        nc.tensor.matmul(
                            ps[:, 0:256],
                            gtb[kt][:, it, :],
                            wo_sb[:, kt, :],
                            start=(kt == 0), stop=(kt == 3),
                        )
                    yt = youtp.tile([128, DOUT], F32, tag="yt")
                    nc.vector.tensor_mul(yt, ps[:, 0:256], g2b)
                    nc.vector.tensor_add(yt, yt, b2b)
                    nc.sync.dma_start(out=out[128 * it:128 * (it + 1), :],
                                      in_=yt)

    _split_excess_waits(nc)
    return nc


def _split_excess_waits(nc):
    """walrus rejects >1 sem-wait per instruction ("Too many sync wait
    commands"); unroll extras into a chain of single-wait same-engine
    NoOps directly before the instruction."""
    ctr = 0
    for fn in nc.m.functions:
        for blk in fn.blocks:
            out = []
            for inst in blk.instructions:
                si = inst.sync_info
                if si is not None and len(si.on_wait) > 1:
                    for w in si.on_wait[:-1]:
                        nop = mybir.InstNoOp(name=f"waitnop-{ctr}")
                        ctr += 1
                        nop.engine = inst.engine
                        nop.sync_info = mybir.SyncInfo(
                            on_wait=[w], on_update=[])
                        out.append(nop)
                    inst.sync_info = mybir.SyncInfo(
                        on_wait=[si.on_wait[-1]], on_update=list(si.on_update))
                out.append(inst)
            blk.instructions = out


_NC_CACHE = None


def kernel(**inputs) -> np.ndarray:
    global _NC_CACHE
    x = np.ascontiguousarray(inputs["x"], dtype=np.float32)        # (8,32,32,256)
    shared = {
        "wq": np.ascontiguousarray(inputs["Wq"], dtype=np.float32),
        "wk": np.ascontiguousarray(inputs["Wk"], dtype=np.float32),
        "wv": np.ascontiguousarray(inputs["Wv"], dtype=np.float32),
        "wo": np.ascontiguousarray(inputs["Wo"], dtype=np.float32),
        "pe": np.ascontiguousarray(inputs["pos_emb"], dtype=np.float32),
        "bo": np.ascontiguousarray(inputs["bo"], dtype=np.float32),
        "gam": np.ascontiguousarray(inputs["gamma"], dtype=np.float32),
        "bet": np.ascontiguousarray(inputs["beta"], dtype=np.float32),
    }
    in_maps = []
    for c in range(NCORES):
        m = dict(shared)
        m["x"] = np.ascontiguousarray(x[c].reshape(N, D))
        in_maps.append(m)

    if _NC_CACHE is None:
        _NC_CACHE = build_nc()
    res = run_bass_kernel_spmd(_NC_CACHE, in_maps, core_ids=list(range(NCORES)))
    outs = [res.results[c]["out"].reshape(FM, FM, DOUT) for c in range(NCORES)]
    return np.stack(outs, axis=0)


if __name__ == "__main__":
    nc = build_nc()
    print("build ok")
    from concourse.timeline_sim import TimelineSim
    tl = TimelineSim(nc, trace=False)
    tl.simulate()
    print(f"HW exec time: {tl.time:.0f} ns")


# revision 8
# speedup vs baseline: 1.0489x; 1.0489x over previous
"""Trainium2 Bass kernel for nn_Attention_89172110999574.

Strategy (8 NeuronCores, data parallel — 1 batch element per core):
  - Scores computed TRANSPOSED (ST[j,i] = k_j . q_i), operands bf16.
  - attn@V swapped: lhsT = exp-scores slice [128 j, 128 i] (stationary),
    rhs = [v | 1] [128 j, 65] -> out OT[i-slice, dv|den] in PSUM. N=65 per
    matmul instead of 512 -> attnV PE cost drops ~4x; softmax denominator
    rides as output column 64.
  - Relative-position bias is block-Toeplitz. Two application paths balanced
    across engines:
      * type-A heads (NB..7): identity-matmul adds raw bias strips into the
        score PSUM (spare PE capacity).
      * type-B heads (0..NB-1): exp(bias) table multiplied into exp(scores)
        on DVE (exp(a+b) = exp(a)*exp(b)).
    Strip tables (bf16) built by strided DRAM->SBUF DMAs in consumption
    order.
  - The whole (head, jt) space runs as one flat software pipeline; V
    projections are interleaved into the first 8 stages; attnV trails
    scores/exp with a dynamic lag (starts at 8 while the V-psum pool is
    alive, catches down to 2), so neither PE nor ACT stalls at phase or
    head boundaries.
  - Normalization: per-partition reciprocal of OT column 64 (DVE), applied
    by Pool tensor_scalar into og[i, is, h', dv]; per-head gelu on ACT;
    each head-pair block moved to phase-E layout by ONE hardware
    DMA-transpose (out[p,m,l] = in[l, 128m+p]).
  - Output projection + BatchNorm affine fused on DVE.
"""

import os
import sys

import numpy as np

for _p in ("/opt/trn_rl_repo", "/root/.axon_site/_ro/trn_rl_repo"):
    if os.path.isdir(_p) and _p not in sys.path:
        sys.path.insert(0, _p)

import concourse.bass as bass
import concourse.tile as tile
from concourse import mybir
from concourse.bass_utils import run_bass_kernel_spmd
from concourse.masks import make_identity

N = 1024          # tokens per batch (32*32)
D = 256           # model dim
H = 8             # heads
DK = 32           # head dim (qk)
DV = 64           # head dim (v)
DOUT = 256        # output dim
NCORES = 8
FM = 32           # fmap
SCALE = float(DK) ** -0.5          # 1/sqrt(32)
RS32 = float(np.sqrt(32.0))        # bias premultiplier: bias/scale = sqrt(32)*E
BN_C = float(1.0 / np.sqrt(1.0 + 1e-5))
F32 = mybir.dt.float32
F32R = mybir.dt.float32r
BF16 = mybir.dt.bfloat16

# heads 0..NB-1 use the exp(bias)-multiply path (DVE); heads NB..7 use
# PE identity-matmul bias adds. Balances PE vs ACT vs DVE busy time.
NB = 4
NA = H - NB


def build_nc():
    nc = bass.Bass("TRN2", target_bir_lowering=False, debug=False)

    # x declared f32r (bit-identical to f32) so transposes run all-f32r
    x = nc.dram_tensor("x", [N, D], F32R, kind="ExternalInput").ap()
    wq = nc.dram_tensor("wq", [D, H * DK], F32, kind="ExternalInput").ap()
    wk = nc.dram_tensor("wk", [D, H * DK], F32, kind="ExternalInput").ap()
    wv = nc.dram_tensor("wv", [D, H * DV], F32, kind="ExternalInput").ap()
    wo = nc.dram_tensor("wo", [H * DV, DOUT], F32, kind="ExternalInput").ap()
    pe = nc.dram_tensor("pe", [N, H], F32, kind="ExternalInput").ap()
    bo = nc.dram_tensor("bo", [DOUT], F32, kind="ExternalInput").ap()
    gam = nc.dram_tensor("gam", [DOUT], F32, kind="ExternalInput").ap()
    bet = nc.dram_tensor("bet", [DOUT], F32, kind="ExternalInput").ap()
    out = nc.dram_tensor("out", [N, DOUT], F32, kind="ExternalOutput").ap()

    # scratch DRAM: per-head |s-31| expansion of pos_emb rows (both bf16)
    # wfa: raw values (type-A heads); wfb: exp(sqrt32*E) (type-B heads)
    wfa = nc.dram_tensor("wfa", [NA, 32, 63], BF16).ap()
    wfb = nc.dram_tensor("wfb", [NB, 32, 63], BF16).ap()

    with tile.TileContext(nc) as tc:
        with (
            tc.tile_pool(name="const", bufs=1) as constp,
            tc.tile_pool(name="big", bufs=1) as bigp,
            tc.tile_pool(name="xin", bufs=2) as xinp,
            tc.tile_pool(name="exps", bufs=4) as expp,
            tc.tile_pool(name="exps2", bufs=10) as exp2p,
            tc.tile_pool(name="small", bufs=1) as smallp,
            tc.tile_pool(name="yout", bufs=3) as youtp,
            tc.tile_pool(name="ps2", bufs=2, space="PSUM") as ps2p,
        ):
            # -------- critical-path DMAs first ---------------------------
            # sync: e_sb (heads the table chain), x-half1, weights
            # scalar: x-half0 (+ table dumps later)
            e_sb = smallp.tile([32, 32, 8], F32, tag="e_sb")
            nc.sync.dma_start(
                out=e_sb,
                in_=bass.AP(tensor=pe.tensor, offset=0,
                            ap=[[32 * H, 32], [H, 32], [1, 32 * H // 32]]),
            )
            xa = []
            for half in range(2):
                xt_in = xinp.tile([128, 4, D], F32R, tag=f"xa{half}", bufs=1,
                                  name=f"xa{half}")
                eng = nc.scalar if half == 0 else nc.sync
                eng.dma_start(
                    out=xt_in,
                    in_=bass.AP(tensor=x.tensor, offset=half * 512 * D,
                                ap=[[D, 128], [128 * D, 4], [1, D]]))
                xa.append(xt_in)
            wq_sb = constp.tile([128, 2, 256], BF16)
            wk_sb = constp.tile([128, 2, 256], BF16)
            wv_sb = constp.tile([128, 2, 512], BF16)
            wo_sb = constp.tile([128, 4, 256], BF16)
            wstgs = []
            for i, (wsrc, nk, cols) in enumerate(
                    ((wq, 2, 256), (wk, 2, 256), (wv, 2, 512), (wo, 4, 256))):
                wstg = xinp.tile([128, nk, cols], F32, tag=f"wstg{i}", bufs=1,
                                 name=f"wstg{i}")
                nc.sync.dma_start(
                    out=wstg,
                    in_=bass.AP(tensor=wsrc.tensor, offset=0,
                                ap=[[cols, 128], [128 * cols, nk], [1, cols]]))
                wstgs.append(wstg)

            # -------- identity (Pool) + weight casts (Pool, early) -------
            ident = constp.tile([128, 128], F32)
            make_identity(nc, ident)
            identr = constp.tile([128, 128], F32R)
            nc.scalar.activation(identr, ident,
                                 mybir.ActivationFunctionType.Copy)
            for dst_sb, wstg in zip((wq_sb, wk_sb, wv_sb, wo_sb), wstgs):
                nc.gpsimd.tensor_copy(dst_sb, wstg)

            # -------- pos-emb row tables ---------------------------------
            # s-flip on DVE: wrow[a, h, s] = E[a, |s-31|, h]
            wrow = smallp.tile([32, 8, 63], F32, tag="wrow")
            nc.vector.tensor_copy(
                wrow[:, :, 0:31],
                bass.AP(tensor=e_sb.tensor, offset=e_sb.offset + 31 * 8,
                        ap=[e_sb.ap[0], [1, 8], [-8, 31]]),
            )
            nc.vector.tensor_copy(
                wrow[:, :, 31:63],
                bass.AP(tensor=e_sb.tensor, offset=e_sb.offset,
                        ap=[e_sb.ap[0], [1, 8], [8, 32]]),
            )
            # type-B rows: exp(sqrt32 * E) bf16; type-A rows: raw bf16
            ewrow = smallp.tile([32, NB, 63], BF16, tag="ewrow")
            nc.scalar.activation(ewrow, wrow[:, 0:NB, :],
                                 mybir.ActivationFunctionType.Exp,
                                 scale=RS32)
            wrowb = smallp.tile([32, NA, 63], BF16, tag="wrowb")
            nc.scalar.activation(wrowb, wrow[:, NB:H, :],
                                 mybir.ActivationFunctionType.Copy)
            nc.scalar.dma_start(out=wfb, in_=ewrow)
            nc.scalar.dma_start(out=wfa, in_=wrowb)

            # strip tables: table[32g+cj, h, u, ci] = row_h[|u-31-g|, |ci-cj|]
            emB = bigp.tile([128, NB, 66, 32], BF16)
            msA = bigp.tile([128, NA, 66, 32], BF16)

            def fill_strip(dst, src_t, h, engs):
                # ascending halves first: the jt=0 slice only needs u >= 31
                for g in range(4):
                    engs[g % len(engs)].dma_start(
                        out=dst[32 * g:32 * (g + 1), h, 31 + g:63 + g, :],
                        in_=bass.AP(tensor=src_t, offset=2016 * h + 31,
                                    ap=[[-1, 32], [63, 32], [1, 32]]),
                    )
                for g in range(4):
                    engs[(g + 1) % len(engs)].dma_start(
                        out=dst[32 * g:32 * (g + 1), h, g:31 + g, :],
                        in_=bass.AP(tensor=src_t,
                                    offset=2016 * h + 31 * 63 + 31,
                                    ap=[[-1, 32], [-63, 31], [1, 32]]),
                    )

            for hb in range(NB):
                fill_strip(emB, wfb.tensor, hb,
                           [nc.scalar, nc.sync] if hb == 0 else [nc.sync])
            for ha in range(NA - 1):
                fill_strip(msA, wfa.tensor, ha, [nc.sync])
            # last type-A head's fills go on Pool's SWDGE, interleaved later

            # v augmented with a ones column per head for the softmax
            # denominator (becomes OT column 64)
            va = bigp.tile([128, 8, 8, 65], BF16)
            nc.scalar.activation(va[:, :, :, 64:65],
                                 ident[:, 0:64],
                                 mybir.ActivationFunctionType.Copy,
                                 bias=1.0, scale=0.0)

            # identity scaled by 32 = 1/scale^2 (bf16; type-A bias adds)
            isc = constp.tile([128, 128], BF16)
            nc.scalar.mul(isc, ident, float(DK))

            xT = bigp.tile([128, 2, N], BF16)
            qT = bigp.tile([128, 2, N], BF16)
            kT = bigp.tile([128, 2, N], BF16)

            # og per head-pair: [128 i-low, 8 is, 2 h', 64 dv] bf16; after
            # gelu, ONE dma transpose -> gT band [128 hv, 8 is, 128 i-low]
            ogt = []
            gtb = []
            for p in range(4):
                og_p = bigp.tile([128, 8, 2, 64], BF16, tag=f"og{p}",
                                 name=f"og{p}")
                gt_p = bigp.tile([128, 8, 128], BF16, tag=f"gt{p}",
                                 name=f"gt{p}")
                ogt.append(og_p)
                gtb.append(gt_p)
            rd = constp.tile([128, 8, 8], F32)  # reciprocal denominators

            with tc.tile_pool(name="psA", bufs=2, space="PSUM") as psAp:
                # ------------- phase A: x -> xT -------------------------
                for half in range(2):
                    for sub in range(4):
                        nt = 4 * half + sub
                        for dt_ in range(2):
                            pst = psAp.tile([128, 128], F32R, tag="psa")
                            nc.tensor.transpose(
                                pst,
                                xa[half][:, sub, 128 * dt_:128 * (dt_ + 1)],
                                identr)
                            nc.vector.tensor_copy(
                                xT[:, dt_, 128 * nt:128 * (nt + 1)], pst)

            NSTAGE = H * 8
            es_q = [None] * NSTAGE   # attnV lhsT operand per stage
            ot_tiles = {}            # head -> (ota, otb)
            drained = [False] * H

            def drain_head(hv):
                # reciprocal + Pool normalize + per-head gelu; dma-transpose
                # per pair once both heads are in og
                ota, otb = ot_tiles.pop(hv)
                pr = hv // 2
                hq = hv % 2
                nc.vector.reciprocal(
                    rd[:, hv, 0:4],
                    ota[:, :, 64:65].rearrange("p a b -> p (a b)"))
                nc.vector.reciprocal(
                    rd[:, hv, 4:8],
                    otb[:, :, 64:65].rearrange("p a b -> p (a b)"))
                for isl in range(8):
                    ot = ota if isl < 4 else otb
                    nc.gpsimd.tensor_scalar_mul(
                        ogt[pr][:, isl, hq, :],
                        ot[:, isl % 4, 0:64],
                        rd[:, hv, isl:isl + 1],
                    )
                ogh = ogt[pr][:, :, hq, :]
                nc.scalar.activation(ogh, ogh,
                                     mybir.ActivationFunctionType.Gelu)
                if hq == 1:
                    og2 = ogt[pr].rearrange("p a b c -> p (a b c)")
                    nc.sync.dma_start_transpose(gtb[pr], og2)
                drained[hv] = True

            def emit_attnv(sv):
                hv, jv = divmod(sv, 8)
                if jv == 0:
                    ota = otp.tile([128, 4, 65], F32, tag="ot",
                                   name=f"ota{hv}")
                    otb = otp.tile([128, 4, 65], F32, tag="ot",
                                   name=f"otb{hv}")
                    ot_tiles[hv] = (ota, otb)
                ota, otb = ot_tiles[hv]
                esv = es_q[sv]
                for isl in range(8):
                    ot = ota if isl < 4 else otb
                    nc.tensor.matmul(
                        ot[:, isl % 4, :],
                        esv[:, 128 * isl:128 * (isl + 1)],
                        va[:, jv, hv, :],
                        start=(jv == 0), stop=(jv == 7),
                    )
                if jv == 7:
                    drain_head(hv)

            def emit_stage(s, vproj_pool):
                h, jt = divmod(s, 8)
                typeB = h < NB
                mtk = h // 4
                pb = 32 * (h % 4)
                ps = ps2p.tile([128, 1024], F32, tag="st")
                for ic in range(2):
                    nc.tensor.matmul(
                        ps[:, 512 * ic:512 * (ic + 1)],
                        kT[pb:pb + 32, mtk, 128 * jt:128 * (jt + 1)],
                        qT[pb:pb + 32, mtk, 512 * ic:512 * (ic + 1)],
                        start=True, stop=typeB,
                        tile_position=(pb, 0),
                    )
                    if not typeB:
                        u0 = 16 * ic + 31 - 4 * jt
                        nc.tensor.matmul(
                            ps[:, 512 * ic:512 * (ic + 1)],
                            isc,
                            msA[:, h - NB, u0:u0 + 16, :],
                            start=False, stop=True,
                        )
                # V projection for token chunk `s` rides stages 0..7
                if vproj_pool is not None:
                    vps = vproj_pool.tile([128, 512], F32, tag="psv")
                    for kt in range(2):
                        nc.tensor.matmul(
                            vps,
                            xT[:, kt, 128 * s:128 * (s + 1)],
                            wv_sb[:, kt, :],
                            start=(kt == 0), stop=(kt == 1),
                        )
                    psr = vps.rearrange("p (h v) -> p h v", v=64)
                    nc.gpsimd.tensor_copy(va[:, s, :, 0:64], psr)
                es = expp.tile([128, 1024], BF16, tag="es")
                nc.scalar.activation(es, ps,
                                     mybir.ActivationFunctionType.Exp,
                                     scale=SCALE)
                if typeB:
                    es2 = exp2p.tile([128, 32, 32], BF16, tag="es2")
                    nc.vector.tensor_mul(
                        es2,
                        es.rearrange("p (a b) -> p a b", b=32),
                        emB[:, h, 31 - 4 * jt:63 - 4 * jt, :],
                    )
                    es_q[s] = es2.rearrange("p a b -> p (a b)")
                else:
                    es_q[s] = es

            # stages 0..7: qk projections + scores with V interleaved; attnV
            # deferred (the V psum pool still owns 2 banks)
            with tc.tile_pool(name="psV", bufs=2, space="PSUM") as psVp:
                for dst_sb, w_sb in ((qT, wq_sb), (kT, wk_sb)):
                    for mt in range(2):
                        for ic in range(2):
                            ps = psVp.tile([128, 512], F32, tag="psv")
                            for kt in range(2):
                                nc.tensor.matmul(
                                    ps,
                                    w_sb[:, kt, 128 * mt:128 * (mt + 1)],
                                    xT[:, kt, 512 * ic:512 * (ic + 1)],
                                    start=(kt == 0), stop=(kt == 1),
                                )
                            nc.vector.tensor_copy(
                                dst_sb[:, mt, 512 * ic:512 * (ic + 1)], ps)
                for s in range(8):
                    emit_stage(s, psVp)

            # stages 8+: attnV catches up (2 per stage) at steady lag 2
            with tc.tile_pool(name="otp", bufs=4, space="PSUM") as otp:
                next_sv = 0
                for s in range(8, NSTAGE + 10):
                    if s < NSTAGE:
                        emit_stage(s, None)
                        # spread the last type-A head's table fills on Pool
                        if 10 <= s < 26 and s % 2 == 0:
                            i = (s - 10) // 2
                            g = i % 4
                            if i < 4:
                                nc.gpsimd.dma_start(
                                    out=msA[32 * g:32 * (g + 1), NA - 1,
                                            31 + g:63 + g, :],
                                    in_=bass.AP(
                                        tensor=wfa.tensor,
                                        offset=2016 * (NA - 1) + 31,
                                        ap=[[-1, 32], [63, 32], [1, 32]]))
                            else:
                                nc.gpsimd.dma_start(
                                    out=msA[32 * g:32 * (g + 1), NA - 1,
                                            g:31 + g, :],
                                    in_=bass.AP(
                                        tensor=wfa.tensor,
                                        offset=2016 * (NA - 1) + 31 * 63 + 31,
                                        ap=[[-1, 32], [-63, 31], [1, 32]]))
                    budget = 2
                    while next_sv <= s - 2 and next_sv < NSTAGE and budget:
                        emit_attnv(next_sv)
                        next_sv += 1
                        budget -= 1

                # ------------- phase E: out proj + BN -------------------
                g2b = constp.tile([128, DOUT], F32)
                b2b = constp.tile([128, DOUT], F32)
                tmpb = constp.tile([128, DOUT], F32)
                nc.sync.dma_start(
                    out=g2b, in_=bass.AP(tensor=gam.tensor, offset=0,
                                         ap=[[0, 128], [1, DOUT]]))
                nc.sync.dma_start(
                    out=b2b, in_=bass.AP(tensor=bet.tensor, offset=0,
                                         ap=[[0, 128], [1, DOUT]]))
                nc.sync.dma_start(
                    out=tmpb, in_=bass.AP(tensor=bo.tensor, offset=0,
                                          ap=[[0, 128], [1, DOUT]]))
                nc.scalar.mul(g2b, g2b, BN_C)
                nc.vector.tensor_mul(tmpb, tmpb, g2b)
                nc.vector.tensor_add(b2b, b2b, tmpb)

                for it in range(8):
                    ps = ps2p.tile([128, 1024], F32, tag="st")
                    for kt in range(4):
                        nc.tensor.matmul(
                            ps[:, 0:256],
                            gtb[kt][:, it, :],
                            wo_sb[:, kt, :],
                            start=(kt == 0), stop=(kt == 3),
                        )
                    yt = youtp.tile([128, DOUT], F32, tag="yt")
                    nc.vector.tensor_mul(yt, ps[:, 0:256], g2b)
                    nc.vector.tensor_add(yt, yt, b2b)
                    nc.sync.dma_start(out=out[128 * it:128 * (it + 1), :],
                                      in_=yt)

    _split_excess_waits(nc)
    return nc


def _split_excess_waits(nc):
    """walrus rejects >1 sem-wait per instruction ("Too many sync wait
    commands"); unroll extras into a chain of single-wait same-engine
    NoOps directly before the instruction."""
    ctr = 0
    for fn in nc.m.functions:
        for blk in fn.blocks:
            out = []
            for inst in blk.instructions:
                si = inst.sync_info
                if si is not None and len(si.on_wait) > 1:
                    for w in si.on_wait[:-1]:
                        nop = mybir.InstNoOp(name=f"waitnop-{ctr}")
                        ctr += 1
                        nop.engine = inst.engine
                        nop.sync_info = mybir.SyncInfo(
                            on_wait=[w], on_update=[])
                        out.append(nop)
                    inst.sync_info = mybir.SyncInfo(
                        on_wait=[si.on_wait[-1]], on_update=list(si.on_update))
                out.append(inst)
            blk.instructions = out


_NC_CACHE = None


def kernel(**inputs) -> np.ndarray:
    global _NC_CACHE
    x = np.ascontiguousarray(inputs["x"], dtype=np.float32)        # (8,32,32,256)
    shared = {
        "wq": np.ascontiguousarray(inputs["Wq"], dtype=np.float32),
        "wk": np.ascontiguousarray(inputs["Wk"], dtype=np.float32),
        "wv": np.ascontiguousarray(inputs["Wv"], dtype=np.float32),
        "wo": np.ascontiguousarray(inputs["Wo"], dtype=np.float32),
        "pe": np.ascontiguousarray(inputs["pos_emb"], dtype=np.float32),
        "bo": np.ascontiguousarray(inputs["bo"], dtype=np.float32),
        "gam": np.ascontiguousarray(inputs["gamma"], dtype=np.float32),
        "bet": np.ascontiguousarray(inputs["beta"], dtype=np.float32),
    }
    in_maps = []
    for c in range(NCORES):
        m = dict(shared)
        m["x"] = np.ascontiguousarray(x[c].reshape(N, D))
        in_maps.append(m)

    if _NC_CACHE is None:
        _NC_CACHE = build_nc()
    res = run_bass_kernel_spmd(_NC_CACHE, in_maps, core_ids=list(range(NCORES)))
    outs = [res.results[c]["out"].reshape(FM, FM, DOUT) for c in range(NCORES)]
    return np.stack(outs, axis=0)


if __name__ == "__main__":
    nc = build_nc()
    print("build ok")
    from concourse.timeline_sim import TimelineSim
    tl = TimelineSim(nc, trace=False)
    tl.simulate()
    print(f"HW exec time: {tl.time:.0f} ns")


# revision 13
# speedup vs baseline: 1.0676x; 1.0179x over previous
"""Trainium2 Bass kernel for nn_Attention_89172110999574.

Strategy (8 NeuronCores, data parallel — 1 batch element per core):
  - Scores computed TRANSPOSED (ST[j,i] = k_j . q_i), operands bf16.
  - attn@V swapped: lhsT = exp-scores slice [128 j, 128 i] (stationary),
    rhs = [v | 1] [128 j, 65] -> out OT[i-slice, dv|den] in PSUM. N=65 per
    matmul instead of 512 -> attnV PE cost drops ~4x; softmax denominator
    rides as output column 64.
  - Relative-position bias is block-Toeplitz. Two application paths balanced
    across engines:
      * type-A heads (NB..7): identity-matmul adds raw bias strips into the
        score PSUM (spare PE capacity).
      * type-B heads (0..NB-1): exp(bias) table multiplied into exp(scores)
        on DVE (exp(a+b) = exp(a)*exp(b)).
    Strip tables (bf16) built by strided DRAM->SBUF DMAs in consumption
    order.
  - The whole (head, jt) space runs as one flat software pipeline; V
    projections are interleaved into the first 8 stages; attnV trails
    scores/exp with a dynamic lag (starts at 8 while the V-psum pool is
    alive, catches down to 2), so neither PE nor ACT stalls at phase or
    head boundaries.
  - Normalization: per-partition reciprocal of OT column 64 (DVE), applied
    by Pool tensor_scalar into og[i, is, h', dv]; per-head gelu on ACT;
    each head-pair block moved to phase-E layout by ONE hardware
    DMA-transpose (out[p,m,l] = in[l, 128m+p]).
  - Output projection + BatchNorm affine fused on DVE.
"""

import os
import sys

import numpy as np

for _p in ("/opt/trn_rl_repo", "/root/.axon_site/_ro/trn_rl_repo"):
    if os.path.isdir(_p) and _p not in sys.path:
        sys.path.insert(0, _p)

import concourse.bass as bass
import concourse.tile as tile
from concourse import mybir
from concourse.bass_utils import run_bass_kernel_spmd
from concourse.masks import make_identity

N = 1024          # tokens per batch (32*32)
D = 256           # model dim
H = 8             # heads
DK = 32           # head dim (qk)
DV = 64           # head dim (v)
DOUT = 256        # output dim
NCORES = 8
FM = 32           # fmap
SCALE = float(DK) ** -0.5          # 1/sqrt(32)
RS32 = float(np.sqrt(32.0))        # bias premultiplier: bias/scale = sqrt(32)*E
BN_C = float(1.0 / np.sqrt(1.0 + 1e-5))
F32 = mybir.dt.float32
F32R = mybir.dt.float32r
BF16 = mybir.dt.bfloat16

# heads 0..NB-1 use the exp(bias)-multiply path (DVE); heads NB..7 use
# PE identity-matmul bias adds. Balances PE vs ACT vs DVE busy time.
NB = 4
NA = H - NB


def build_nc():
    nc = bass.Bass("TRN2", target_bir_lowering=False, debug=False)

    # x declared f32r (bit-identical to f32) so transposes run all-f32r
    x = nc.dram_tensor("x", [N, D], F32R, kind="ExternalInput").ap()
    wq = nc.dram_tensor("wq", [D, H * DK], F32, kind="ExternalInput").ap()
    wk = nc.dram_tensor("wk", [D, H * DK], F32, kind="ExternalInput").ap()
    wv = nc.dram_tensor("wv", [D, H * DV], F32, kind="ExternalInput").ap()
    wo = nc.dram_tensor("wo", [H * DV, DOUT], F32, kind="ExternalInput").ap()
    pe = nc.dram_tensor("pe", [N, H], F32, kind="ExternalInput").ap()
    bo = nc.dram_tensor("bo", [DOUT], F32, kind="ExternalInput").ap()
    gam = nc.dram_tensor("gam", [DOUT], F32, kind="ExternalInput").ap()
    bet = nc.dram_tensor("bet", [DOUT], F32, kind="ExternalInput").ap()
    out = nc.dram_tensor("out", [N, DOUT], F32, kind="ExternalOutput").ap()

    # scratch DRAM: per-head |s-31| expansion of pos_emb rows (both bf16)
    # wfa: raw values (type-A heads); wfb: exp(sqrt32*E) (type-B heads)
    wfa = nc.dram_tensor("wfa", [NA, 32, 63], BF16).ap()
    wfb = nc.dram_tensor("wfb", [NB, 32, 63], BF16).ap()

    with tile.TileContext(nc) as tc:
        with (
            tc.tile_pool(name="const", bufs=1) as constp,
            tc.tile_pool(name="big", bufs=1) as bigp,
            tc.tile_pool(name="xin", bufs=2) as xinp,
            tc.tile_pool(name="exps", bufs=4) as expp,
            tc.tile_pool(name="exps2", bufs=10) as exp2p,
            tc.tile_pool(name="small", bufs=1) as smallp,
            tc.tile_pool(name="yout", bufs=3) as youtp,
            tc.tile_pool(name="ps2", bufs=2, space="PSUM") as ps2p,
        ):
            # -------- critical-path DMAs first ---------------------------
            # sync: e_sb (heads the table chain), x-half1, weights
            # scalar: x-half0 (+ table dumps later)
            e_sb = smallp.tile([32, 32, 8], F32, tag="e_sb")
            nc.sync.dma_start(
                out=e_sb,
                in_=bass.AP(tensor=pe.tensor, offset=0,
                            ap=[[32 * H, 32], [H, 32], [1, 32 * H // 32]]),
            )
            xa = []
            for half in range(2):
                xt_in = xinp.tile([128, 4, D], F32R, tag=f"xa{half}", bufs=1,
                                  name=f"xa{half}")
                eng = nc.scalar if half == 0 else nc.sync
                eng.dma_start(
                    out=xt_in,
                    in_=bass.AP(tensor=x.tensor, offset=half * 512 * D,
                                ap=[[D, 128], [128 * D, 4], [1, D]]))
                xa.append(xt_in)
            wq_sb = constp.tile([128, 2, 256], BF16)
            wk_sb = constp.tile([128, 2, 256], BF16)
            wv_sb = constp.tile([128, 2, 512], BF16)
            wo_sb = constp.tile([128, 4, 256], BF16)
            wstgs = []
            for i, (wsrc, nk, cols) in enumerate(
                    ((wq, 2, 256), (wk, 2, 256), (wv, 2, 512), (wo, 4, 256))):
                wstg = xinp.tile([128, nk, cols], F32, tag=f"wstg{i}", bufs=1,
                                 name=f"wstg{i}")
                nc.sync.dma_start(
                    out=wstg,
                    in_=bass.AP(tensor=wsrc.tensor, offset=0,
                                ap=[[cols, 128], [128 * cols, nk], [1, cols]]))
                wstgs.append(wstg)

            # -------- identity (Pool) + weight casts (Pool, early) -------
            ident = constp.tile([128, 128], F32)
            make_identity(nc, ident)
            identr = constp.tile([128, 128], F32R)
            nc.scalar.activation(identr, ident,
                                 mybir.ActivationFunctionType.Copy)
            for dst_sb, wstg in zip((wq_sb, wk_sb, wv_sb, wo_sb), wstgs):
                nc.gpsimd.tensor_copy(dst_sb, wstg)

            # -------- pos-emb row tables ---------------------------------
            # s-flip on DVE: wrow[a, h, s] = E[a, |s-31|, h]
            wrow = smallp.tile([32, 8, 63], F32, tag="wrow")
            nc.vector.tensor_copy(
                wrow[:, :, 0:31],
                bass.AP(tensor=e_sb.tensor, offset=e_sb.offset + 31 * 8,
                        ap=[e_sb.ap[0], [1, 8], [-8, 31]]),
            )
            nc.vector.tensor_copy(
                wrow[:, :, 31:63],
                bass.AP(tensor=e_sb.tensor, offset=e_sb.offset,
                        ap=[e_sb.ap[0], [1, 8], [8, 32]]),
            )
            # type-B rows: exp(sqrt32 * E) bf16; type-A rows: raw bf16
            ewrow = smallp.tile([32, NB, 63], BF16, tag="ewrow")
            nc.scalar.activation(ewrow, wrow[:, 0:NB, :],
                                 mybir.ActivationFunctionType.Exp,
                                 scale=RS32)
            wrowb = smallp.tile([32, NA, 63], BF16, tag="wrowb")
            nc.scalar.activation(wrowb, wrow[:, NB:H, :],
                                 mybir.ActivationFunctionType.Copy)
            nc.sync.dma_start(out=wfb, in_=ewrow)
            nc.sync.dma_start(out=wfa, in_=wrowb)

            # strip tables: table[32g+cj, h, u, ci] = row_h[|u-31-g|, |ci-cj|]
            emB = bigp.tile([128, NB, 66, 32], BF16)
            msA = bigp.tile([128, NA, 66, 32], BF16)

            def fill_strip(dst, src_t, h, engs):
                # ascending halves first: the jt=0 slice only needs u >= 31
                for g in range(4):
                    engs[g % len(engs)].dma_start(
                        out=dst[32 * g:32 * (g + 1), h, 31 + g:63 + g, :],
                        in_=bass.AP(tensor=src_t, offset=2016 * h + 31,
                                    ap=[[-1, 32], [63, 32], [1, 32]]),
                    )
                for g in range(4):
                    engs[(g + 1) % len(engs)].dma_start(
                        out=dst[32 * g:32 * (g + 1), h, g:31 + g, :],
                        in_=bass.AP(tensor=src_t,
                                    offset=2016 * h + 31 * 63 + 31,
                                    ap=[[-1, 32], [-63, 31], [1, 32]]),
                    )

            for hb in range(NB):
                fill_strip(emB, wfb.tensor, hb, [nc.sync])
            for ha in range(NA):
                fill_strip(msA, wfa.tensor, ha, [nc.sync])

            # BN affine rows (partition-broadcast straight from DRAM):
            # g2 = gamma*c ; b2 = bo*g2 + beta   (all compute on DVE)
            g2b = constp.tile([128, DOUT], F32)
            b2b = constp.tile([128, DOUT], F32)
            tmpb = constp.tile([128, DOUT], F32)
            nc.sync.dma_start(
                out=g2b, in_=bass.AP(tensor=gam.tensor, offset=0,
                                     ap=[[0, 128], [1, DOUT]]))
            nc.sync.dma_start(
                out=b2b, in_=bass.AP(tensor=bet.tensor, offset=0,
                                     ap=[[0, 128], [1, DOUT]]))
            nc.sync.dma_start(
                out=tmpb, in_=bass.AP(tensor=bo.tensor, offset=0,
                                      ap=[[0, 128], [1, DOUT]]))
            nc.vector.tensor_scalar_mul(g2b, g2b, BN_C)
            nc.vector.tensor_mul(tmpb, tmpb, g2b)
            nc.vector.tensor_add(b2b, b2b, tmpb)

            # v augmented with a ones column per head for the softmax
            # denominator (becomes OT column 64)
            va = bigp.tile([128, 8, 8, 65], BF16)
            nc.scalar.activation(va[:, :, :, 64:65],
                                 ident[:, 0:64],
                                 mybir.ActivationFunctionType.Copy,
                                 bias=1.0, scale=0.0)

            # identity scaled by 32 = 1/scale^2 (bf16; type-A bias adds)
            isc = constp.tile([128, 128], BF16)
            nc.scalar.mul(isc, ident, float(DK))

            xT = bigp.tile([128, 2, N], BF16)
            qT = bigp.tile([128, 2, N], BF16)
            kT = bigp.tile([128, 2, N], BF16)

            # og per head-pair: [128 i-low, 8 is, 2 h', 64 dv] bf16; after
            # gelu, ONE dma transpose -> gT band [128 hv, 8 is, 128 i-low]
            ogt = []
            gtb = []
            for p in range(4):
                og_p = bigp.tile([128, 8, 2, 64], BF16, tag=f"og{p}",
                                 name=f"og{p}")
                gt_p = bigp.tile([128, 8, 128], BF16, tag=f"gt{p}",
                                 name=f"gt{p}")
                ogt.append(og_p)
                gtb.append(gt_p)
            rd = constp.tile([128, 8, 8], F32)  # reciprocal denominators

            with tc.tile_pool(name="psA", bufs=2, space="PSUM") as psAp:
                # ------------- phase A: x -> xT -------------------------
                for half in range(2):
                    for sub in range(4):
                        nt = 4 * half + sub
                        for dt_ in range(2):
                            pst = psAp.tile([128, 128], F32R, tag="psa")
                            nc.tensor.transpose(
                                pst,
                                xa[half][:, sub, 128 * dt_:128 * (dt_ + 1)],
                                identr)
                            nc.vector.tensor_copy(
                                xT[:, dt_, 128 * nt:128 * (nt + 1)], pst)

            NSTAGE = H * 8
            es_q = [None] * NSTAGE   # attnV lhsT operand per stage
            ot_tiles = {}            # head -> (ota, otb)
            drained = [False] * H

            def drain_head(hv):
                # reciprocal + Pool normalize + per-head gelu; dma-transpose
                # per pair once both heads are in og
                ota, otb = ot_tiles.pop(hv)
                pr = hv // 2
                hq = hv % 2
                nc.vector.reciprocal(
                    rd[:, hv, 0:4],
                    ota[:, :, 64:65].rearrange("p a b -> p (a b)"))
                nc.vector.reciprocal(
                    rd[:, hv, 4:8],
                    otb[:, :, 64:65].rearrange("p a b -> p (a b)"))
                for isl in range(8):
                    ot = ota if isl < 4 else otb
                    nc.gpsimd.tensor_scalar_mul(
                        ogt[pr][:, isl, hq, :],
                        ot[:, isl % 4, 0:64],
                        rd[:, hv, isl:isl + 1],
                    )
                ogh = ogt[pr][:, :, hq, :]
                nc.scalar.activation(ogh, ogh,
                                     mybir.ActivationFunctionType.Gelu)
                if hq == 1:
                    og2 = ogt[pr].rearrange("p a b c -> p (a b c)")
                    nc.sync.dma_start_transpose(gtb[pr], og2)
                drained[hv] = True

            def emit_attnv(sv):
                hv, jv = divmod(sv, 8)
                if jv == 0:
                    ota = otp.tile([128, 4, 65], F32, tag="ot",
                                   name=f"ota{hv}")
                    otb = otp.tile([128, 4, 65], F32, tag="ot",
                                   name=f"otb{hv}")
                    ot_tiles[hv] = (ota, otb)
                ota, otb = ot_tiles[hv]
                esv = es_q[sv]
                for isl in range(8):
                    ot = ota if isl < 4 else otb
                    nc.tensor.matmul(
                        ot[:, isl % 4, :],
                        esv[:, 128 * isl:128 * (isl + 1)],
                        va[:, jv, hv, :],
                        start=(jv == 0), stop=(jv == 7),
                    )
                if jv == 7:
                    drain_head(hv)

            def emit_stage(s, vproj_pool):
                h, jt = divmod(s, 8)
                typeB = h < NB
                mtk = h // 4
                pb = 32 * (h % 4)
                ps = ps2p.tile([128, 1024], F32, tag="st")
                for ic in range(2):
                    nc.tensor.matmul(
                        ps[:, 512 * ic:512 * (ic + 1)],
                        kT[pb:pb + 32, mtk, 128 * jt:128 * (jt + 1)],
                        qT[pb:pb + 32, mtk, 512 * ic:512 * (ic + 1)],
                        start=True, stop=typeB,
                        tile_position=(pb, 0),
                    )
                    if not typeB:
                        u0 = 16 * ic + 31 - 4 * jt
                        nc.tensor.matmul(
                            ps[:, 512 * ic:512 * (ic + 1)],
                            isc,
                            msA[:, h - NB, u0:u0 + 16, :],
                            start=False, stop=True,
                        )
                # V projection for token chunk `s` rides stages 0..7
                if vproj_pool is not None:
                    vps = vproj_pool.tile([128, 512], F32, tag="psv")
                    for kt in range(2):
                        nc.tensor.matmul(
                            vps,
                            xT[:, kt, 128 * s:128 * (s + 1)],
                            wv_sb[:, kt, :],
                            start=(kt == 0), stop=(kt == 1),
                        )
                    psr = vps.rearrange("p (h v) -> p h v", v=64)
                    nc.gpsimd.tensor_copy(va[:, s, :, 0:64], psr)
                es = expp.tile([128, 1024], BF16, tag="es")
                nc.scalar.activation(es, ps,
                                     mybir.ActivationFunctionType.Exp,
                                     scale=SCALE)
                if typeB:
                    es2 = exp2p.tile([128, 32, 32], BF16, tag="es2")
                    nc.vector.tensor_mul(
                        es2,
                        es.rearrange("p (a b) -> p a b", b=32),
                        emB[:, h, 31 - 4 * jt:63 - 4 * jt, :],
                    )
                    es_q[s] = es2.rearrange("p a b -> p (a b)")
                else:
                    es_q[s] = es

            # stages 0..7: qk projections + scores with V interleaved; attnV
            # deferred (the V psum pool still owns 2 banks)
            with tc.tile_pool(name="psV", bufs=2, space="PSUM") as psVp:
                for dst_sb, w_sb in ((qT, wq_sb), (kT, wk_sb)):
                    for mt in range(2):
                        for ic in range(2):
                            ps = psVp.tile([128, 512], F32, tag="psv")
                            for kt in range(2):
                                nc.tensor.matmul(
                                    ps,
                                    w_sb[:, kt, 128 * mt:128 * (mt + 1)],
                                    xT[:, kt, 512 * ic:512 * (ic + 1)],
                                    start=(kt == 0), stop=(kt == 1),
                                )
                            nc.vector.tensor_copy(
                                dst_sb[:, mt, 512 * ic:512 * (ic + 1)], ps)
                for s in range(8):
                    emit_stage(s, psVp)

            # stages 8+: attnV catches up (2 per stage) at steady lag 2
            with tc.tile_pool(name="otp", bufs=4, space="PSUM") as otp:
                next_sv = 0
                for s in range(8, NSTAGE + 10):
                    if s < NSTAGE:
                        emit_stage(s, None)
                    budget = 2
                    while next_sv <= s - 2 and next_sv < NSTAGE and budget:
                        emit_attnv(next_sv)
                        next_sv += 1
                        budget -= 1

                # ------------- phase E: out proj + BN -------------------
                for it in range(8):
                    ps = ps2p.tile([128, 1024], F32, tag="st")
                    for kt in range(4):
                        nc.tensor.matmul(
                            ps[:, 0:256],
                            gtb[kt][:, it, :],
                            wo_sb[:, kt, :],
                            start=(kt == 0), stop=(kt == 3),
                        )
                    yt = youtp.tile([128, DOUT], F32, tag="yt")
                    nc.vector.tensor_mul(yt, ps[:, 0:256], g2b)
                    nc.vector.tensor_add(yt, yt, b2b)
                    nc.sync.dma_start(out=out[128 * it:128 * (it + 1), :],
                                      in_=yt)

    _split_excess_waits(nc)
    return nc


def _split_excess_waits(nc):
    """walrus rejects >1 sem-wait per instruction ("Too many sync wait
    commands"); unroll extras into a chain of single-wait same-engine
    NoOps directly before the instruction."""
    ctr = 0
    for fn in nc.m.functions:
        for blk in fn.blocks:
            out = []
            for inst in blk.instructions:
                si = inst.sync_info
                if si is not None and len(si.on_wait) > 1:
                    for w in si.on_wait[:-1]:
                        nop = mybir.InstNoOp(name=f"waitnop-{ctr}")
                        ctr += 1
                        nop.engine = inst.engine
                        nop.sync_info = mybir.SyncInfo(
                            on_wait=[w], on_update=[])
                        out.append(nop)
                    inst.sync_info = mybir.SyncInfo(
                        on_wait=[si.on_wait[-1]], on_update=list(si.on_update))
                out.append(inst)
            blk.instructions = out


_NC_CACHE = None


def kernel(**inputs) -> np.ndarray:
    global _NC_CACHE
    x = np.ascontiguousarray(inputs["x"], dtype=np.float32)        # (8,32,32,256)
    shared = {
        "wq": np.ascontiguousarray(inputs["Wq"], dtype=np.float32),
        "wk": np.ascontiguousarray(inputs["Wk"], dtype=np.float32),
        "wv": np.ascontiguousarray(inputs["Wv"], dtype=np.float32),
        "wo": np.ascontiguousarray(inputs["Wo"], dtype=np.float32),
        "pe": np.ascontiguousarray(inputs["pos_emb"], dtype=np.float32),
        "bo": np.ascontiguousarray(inputs["bo"], dtype=np.float32),
        "gam": np.ascontiguousarray(inputs["gamma"], dtype=np.float32),
        "bet": np.ascontiguousarray(inputs["beta"], dtype=np.float32),
    }
    in_maps = []
    for c in range(NCORES):
        m = dict(shared)
        m["x"] = np.ascontiguousarray(x[c].reshape(N, D))
        in_maps.append(m)

    if _NC_CACHE is None:
        _NC_CACHE = build_nc()
    res = run_bass_kernel_spmd(_NC_CACHE, in_maps, core_ids=list(range(NCORES)))
    outs = [res.results[c]["out"].reshape(FM, FM, DOUT) for c in range(NCORES)]
    return np.stack(outs, axis=0)


if __name__ == "__main__":
    nc = build_nc()
    print("build ok")
    from concourse.timeline_sim import TimelineSim
    tl = TimelineSim(nc, trace=False)
    tl.simulate()
    print(f"HW exec time: {tl.time:.0f} ns")
